# revision 36
# baseline (speedup 1.0000x reference)
"""Causal multi-head attention (B=2, T=2048, D=1024, NH=16, HD=64) on 8 trn2
NeuronCores.

Sharding: data-parallel over batch (2) x tensor-parallel over head groups (4),
Megatron-style. Core c handles batch c//4, heads 4*(c%4)..4*(c%4)+3; the host
sums the 4 partial output projections per batch.

Layout/schedule (vs the O^T baseline, 156.3us -> 131.8us):
- PV runs in NATURAL layout: O[q,hd+1] accumulates with the P^T block as the
  STATIONARY operand (ldweights is engine-free) and V (with a ones column for
  the softmax denominator Z) as the moving operand, so each PV matmul has 128
  output partitions and only 65 moving columns -- half the PE cycles of the
  O^T form (65/128-partition waste eliminated). PSUM start-zeroing is
  bank-granular, so exactly one matmul per O tile carries start=True; the
  other heads' first writes land on its bank-wide pending-zero.
- Normalization becomes per-partition scalars: 1/Z via
  reciprocal_approx_fast on [128,4,1] tiles, applied with tensor_scalar_mul,
  emitted at each q-block's PV *stop* step so O banks recycle early.
- The f-major layout the projection needs is produced by dma_start_transpose
  (XBAR 16x128 tiles, 14ns each on the DMA engines -- off every compute
  engine). Projection then consumes AT^T blocks with wp stationary.
- Activation engine runs the exp stream (the attention-phase co-critical
  resource); psum->sbuf staging goes to DVE, except the tail where Act is
  free. Output DMAs are issued one slot late so their SEQ-level waits never
  block the exp stream on the Act queue.
- qkv is overlapped with early attention: 8 concurrent kt-outer psum groups
  (4 psa + the idle psS slots) track the per-kt x DMA arrivals, then S/exp
  steps interleave with the remaining qkv/V psum groups so Act starts
  exp'ing ~11us in. PVs are deferred until the qkv psum pools close (PSUM
  budget: 8 banks exactly in each phase). The sim's engines execute past
  blocked instructions, so phase B emits all remaining S steps up front and
  lets the psS/pt/O rings self-pace.
- S/exp emission and PV/norm/proj processing both end on a LONG chunk
  (PV_ORDER [1,3,0,2]) whose spread-out norms + early proj halves hide the
  tail normalize/transpose chains.
"""

import sys

if "/opt/trn_rl_repo" not in sys.path:
    sys.path.insert(0, "/opt/trn_rl_repo")

import numpy as np
import concourse.mybir as mybir
from concourse import bacc
from concourse.tile import TileContext
from concourse import bass_utils

B, T, D = 2, 2048, 1024
NH, HD = 16, 64
HL = 4  # heads per core
N_CORES = 8

KT = D // 128  # 8 contraction tiles over model dim
TT = T // 128  # 16 t-blocks of 128

F32 = mybir.dt.float32
BF16 = mybir.dt.bfloat16

QC_ORDER = [1, 3, 2, 0]
DEPTH_B = 4  # phase-B S-emission lookahead (steps) over PV consumption
COOL = 1  # slots between woven proj pieces
OVERLAP_CAP = 52  # max S items emitted during the qkv overlap (pt-pool bound)
OV_SPG = 2  # S items interleaved per qkv/V psum group


def build_nc():
    nc = bacc.Bacc()
    xT = nc.dram_tensor("xT", [D, T], BF16, kind="ExternalInput")
    wqk = nc.dram_tensor("wqk", [D, 512], BF16, kind="ExternalInput")
    wv = nc.dram_tensor("wv", [D, 256], BF16, kind="ExternalInput")
    wp = nc.dram_tensor("wp", [256, D], BF16, kind="ExternalInput")
    tric = nc.dram_tensor("tric", [128, 2, 128], BF16, kind="ExternalInput")
    outT = nc.dram_tensor("outT", [D, T], BF16, kind="ExternalOutput")

    # attention step list: (qc, [(kb, rlo, rhi, slo), ...]); the last two
    # diagonal blocks of each q-chunk are packed into one psum tile / one exp
    steps = []
    for qc in QC_ORDER:
        nkb = 4 * qc + 4
        for kb in range(nkb - 2):
            lo = max(128 * (kb - 4 * qc), 0)
            steps.append((qc, [(kb, lo, 512, lo)]))
        steps.append(
            (qc, [(nkb - 2, 256, 512, 256), (nkb - 1, 128, 256, 384)])
        )
    NS = len(steps)
    # PV/normalize/proj processing order AND phase-B S-emission order: both
    # end on a LONG chunk (its norm-at-stop spreading + early proj halves
    # hide the tail chains). The overlap S order stays QC_ORDER.
    PV_ORDER = [1, 3, 0, 2]
    pv_steps = [
        i for qc in PV_ORDER for i, (q, _) in enumerate(steps) if q == qc
    ]
    LAST_QC = PV_ORDER[-1]

    with TileContext(nc) as tc:
        with (
            tc.tile_pool(name="persist", bufs=1) as pers,
            tc.tile_pool(name="ptile", bufs=52) as ppool,
            tc.tile_pool(name="stage", bufs=1) as stg,
            tc.tile_pool(name="psS", bufs=2, space="PSUM") as pss_pool,
        ):
            qkT_sb = [
                pers.tile([128, T], BF16, tag=f"qkT{mt}", name=f"qkT{mt}")
                for mt in range(4)
            ]
            tri_sb = pers.tile([128, 2, 128], BF16, tag="tri", name="tri")
            V1_sb = [
                pers.tile([128, HL, 65], BF16, tag=f"V1_{tt}", name=f"V1_{tt}")
                for tt in range(TT)
            ]
            # AT natural [q-part, (qb 16, f 256)]; f = (head, hd) per core
            ATn_sb = pers.tile([128, TT, 256], BF16, tag="ATn", name="ATn")
            # AT^T per chunk: blocks (qb-local, f-half) of [128 f, 128 q]
            ATt_sb = [
                pers.tile([128, 4, 2, 128], BF16, tag=f"ATt{qc}", name=f"ATt{qc}")
                for qc in range(4)
            ]
            wp_sb = pers.tile([128, 2, D], BF16, tag="wp", name="wp")

            # ---------------- S/exp emission machinery ----------------
            emitted = set()
            tri_done = set()
            qk_copied = set()
            pt_tiles = {}

            def step_ready(i, p):
                qc, regs = steps[i]
                if (p, qc) not in qk_copied:
                    return False
                maxkb = max(r[0] for r in regs)
                return all(
                    (2 + p, g) in qk_copied for g in range(maxkb // 4 + 1)
                )

            def emit_S(i, p):
                qc, regs = steps[i]
                tlo = min(r[1] for r in regs)
                thi = max(r[2] for r in regs)
                qT = qkT_sb[p]
                kT = qkT_sb[2 + p]
                psS = pss_pool.tile([128, 2, 512], F32, tag="s", name=f"s{p}_{i}")
                pt = ppool.tile([128, 2, 512], BF16, tag="pt", name=f"pt{p}_{i}")
                for kb, rlo, rhi, slo in regs:
                    for hslot in range(2):
                        nc.tensor.matmul(
                            psS[:, hslot, rlo:rhi],
                            kT[
                                64 * hslot : 64 * hslot + 64,
                                kb * 128 : (kb + 1) * 128,
                            ],
                            qT[
                                64 * hslot : 64 * hslot + 64,
                                qc * 512 + slo : qc * 512 + slo + (rhi - rlo),
                            ],
                            start=True,
                            stop=True,
                        )
                nc.scalar.activation(
                    pt[:, :, tlo:thi],
                    psS[:, :, tlo:thi],
                    mybir.ActivationFunctionType.Exp,
                    scale=0.125,
                )
                pt_tiles[(i, p)] = pt
                emitted.add((i, p))

            def emit_tri(i):
                # causal zeroing of the diagonal 128-band; deferred until just
                # before the consuming PV so the DVE never waits on a lagging
                # exp (the exp finished long ago by PV time)
                if i in tri_done or i >= NS:
                    return
                qc, regs = steps[i]
                for p in range(2):
                    pt = pt_tiles[(i, p)]
                    for kb, rlo, rhi, slo in regs:
                        if 128 * kb >= slo + qc * 512:
                            nc.vector.tensor_mul(
                                pt[:, :, rlo : rlo + 128],
                                pt[:, :, rlo : rlo + 128],
                                tri_sb[:, :, 0:128],
                            )
                tri_done.add(i)

            item_queue = [(i, p) for i in range(NS) for p in range(2)]
            n_ov = [0]

            def ov_emit(max_n):
                n = 0
                k = 0
                while (
                    n < max_n
                    and n_ov[0] < OVERLAP_CAP
                    and k < len(item_queue)
                ):
                    it = item_queue[k]
                    if it in emitted:
                        item_queue.pop(k)
                        continue
                    if step_ready(*it):
                        emit_S(*it)
                        item_queue.pop(k)
                        n += 1
                        n_ov[0] += 1
                    else:
                        k += 1
                return n

            # ---------------- phase A: qkv + V, overlapped ----------------
            with tc.tile_pool(name="qkv_in", bufs=1) as qin:
                xT_all = qin.tile([128, KT, T], BF16, tag="xT", name="xT_all")
                wqk_all = qin.tile(
                    [128, KT, 512], BF16, tag="wqk", name="wqk_all"
                )
                wv_all = qin.tile([128, KT, 256], BF16, tag="wv", name="wv_all")

                # load order tuned for the A1 wave: wqk kt0, x first halves
                # (wave1 reads only T-cols 0..1023), wqk rest, x second
                # halves, then V/proj weights. x halves arrive every ~0.7us
                # so the kt-outer wave1 matmuls start ~2.5us in and stay fed.
                # weights-only on the sync queue; all x plus the late
                # weights on the scalar queue -- the HWDGE alternates queues,
                # so this keeps the x first-half stream dense (the kt-outer
                # wave1 is paced by it) and pushes second halves strictly
                # after it
                nc.scalar.dma_start(
                    out=xT_all[:, 0, 0:512], in_=xT[0:128, 0:512]
                )
                nc.sync.dma_start(out=wqk_all[:, 0, :], in_=wqk[0:128, :])
                nc.scalar.dma_start(
                    out=xT_all[:, 0, 512:1024], in_=xT[0:128, 512:1024]
                )
                nc.sync.dma_start(
                    out=wqk_all[:, 1:3, :],
                    in_=wqk[128:384, :].rearrange("(k p) c -> p k c", k=2),
                )
                nc.scalar.dma_start(
                    out=xT_all[:, 1, 0:1024], in_=xT[128:256, 0:1024]
                )
                nc.sync.dma_start(
                    out=wqk_all[:, 3:5, :],
                    in_=wqk[384:640, :].rearrange("(k p) c -> p k c", k=2),
                )
                nc.scalar.dma_start(
                    out=xT_all[:, 2, 0:1024], in_=xT[256:384, 0:1024]
                )
                nc.sync.dma_start(
                    out=wqk_all[:, 5:KT, :],
                    in_=wqk[640:D, :].rearrange("(k p) c -> p k c", k=3),
                )
                for kt in range(3, KT):
                    nc.scalar.dma_start(
                        out=xT_all[:, kt, 0:1024],
                        in_=xT[kt * 128 : (kt + 1) * 128, 0:1024],
                    )
                for kt in range(KT):
                    eng = [nc.sync, nc.scalar][kt % 2]
                    eng.dma_start(
                        out=xT_all[:, kt, 1024:2048],
                        in_=xT[kt * 128 : (kt + 1) * 128, 1024:2048],
                    )
                nc.scalar.dma_start(
                    out=wv_all,
                    in_=wv[:, :].rearrange("(k p) c -> p k c", k=KT),
                )
                for ft in range(2):
                    nc.scalar.dma_start(
                        out=wp_sb[:, ft, :], in_=wp[ft * 128 : (ft + 1) * 128, :]
                    )
                nc.scalar.dma_start(out=tri_sb, in_=tric[:, :, :])
                for tt in range(TT):
                    nc.gpsimd.memset(V1_sb[tt][:, :, 64:65], 1.0)

                with tc.tile_pool(name="psA1", bufs=4, space="PSUM") as psa1:
                    # wave1: ALL m-tiles x T-cols 0..1023, kt-outer so the
                    # matmul stream tracks the per-kt x DMA arrivals. 8
                    # concurrent psum groups: 4 from psa1 plus the 4 halves
                    # of the 2 psS ring slots (idle until the first S step),
                    # so the PE is ~fully busy during the paced x load.
                    wave1 = [(0, 0), (2, 0), (0, 1), (2, 1)]
                    wave1b = [(1, 0), (3, 0), (1, 1), (3, 1)]
                    pstiles = {}
                    for mt, g in wave1:
                        pstiles[(mt, g)] = psa1.tile(
                            [128, 512], F32, tag="qkps", name=f"qk{mt}{g}"
                        )
                    sslots = [
                        pss_pool.tile([128, 2, 512], F32, tag="s", name=f"w1b{i}")
                        for i in range(2)
                    ]
                    for i, (mt, g) in enumerate(wave1b):
                        pstiles[(mt, g)] = sslots[i // 2][:, i % 2, :]
                    for kt in range(KT):
                        for mt, g in wave1 + wave1b:
                            nc.tensor.matmul(
                                pstiles[(mt, g)],
                                wqk_all[:, kt, mt * 128 : (mt + 1) * 128],
                                xT_all[:, kt, g * 512 : (g + 1) * 512],
                                start=(kt == 0),
                                stop=(kt == KT - 1),
                            )
                    for i, (mt, g) in enumerate(wave1 + wave1b):
                        # split copies across Act (idle before the first exp)
                        # and DVE
                        eng = nc.scalar.copy if i % 2 == 0 else (
                            lambda o, s: nc.vector.tensor_copy(o, s)
                        )
                        eng(
                            qkT_sb[mt][:, g * 512 : (g + 1) * 512],
                            pstiles[(mt, g)],
                        )
                        qk_copied.add((mt, g))
                    ov_emit(2)
                    # remaining qk groups, kt-inner, ordered to unlock S
                    # items (qc0 p1 first, then qc3 p0, ...) as fast as Act
                    # can consume them
                    rest = [
                        (0, 3), (2, 2), (2, 3), (3, 2),
                        (3, 3), (1, 3), (0, 2), (1, 2),
                    ]
                    for gi, (mt, g) in enumerate(rest):
                        ps = psa1.tile(
                            [128, 512], F32, tag="qkps", name=f"qk{mt}{g}"
                        )
                        for kt in range(KT):
                            nc.tensor.matmul(
                                ps,
                                wqk_all[:, kt, mt * 128 : (mt + 1) * 128],
                                xT_all[:, kt, g * 512 : (g + 1) * 512],
                                start=(kt == 0),
                                stop=(kt == KT - 1),
                            )
                        nc.vector.tensor_copy(
                            qkT_sb[mt][:, g * 512 : (g + 1) * 512], ps
                        )
                        qk_copied.add((mt, g))
                        ov_emit(1 if gi < 4 else OV_SPG)

                with tc.tile_pool(name="psB", bufs=2, space="PSUM") as psb:
                    for tt in range(TT):
                        psv = psb.tile([128, 256], F32, tag="v", name=f"v{tt}")
                        for kt in range(KT):
                            nc.tensor.matmul(
                                psv,
                                xT_all[:, kt, tt * 128 : (tt + 1) * 128],
                                wv_all[:, kt, :],
                                start=(kt == 0),
                                stop=(kt == KT - 1),
                            )
                        nc.vector.tensor_copy(V1_sb[tt][:, :, 0:64], psv)
                        ov_emit(OV_SPG)

            # ---------------- phase B: PV / normalize / proj ----------------
            pso_tiles = {}

            def emit_PV(j, p):
                qc, regs = steps[j]
                pt = pt_tiles[(j, p)]
                # psum start-zeroing is bank-granular: exactly ONE matmul per
                # O tile (the first to touch the bank: kb==0, h==0) carries
                # start=True. Its bank-wide pending-zero mark makes the other
                # heads' first writes land on zeros, and later accumulates
                # add normally -- no separate memset needed.
                for kb, rlo, rhi, slo in regs:
                    for hslot in range(2):
                        h = 2 * p + hslot
                        for qbl in range(slo // 128, (slo + rhi - rlo) // 128):
                            off = qbl * 128 - slo
                            nc.tensor.matmul(
                                pso_tiles[(qc, qbl)][:, h, :],
                                pt[:, hslot, rlo + off : rlo + off + 128],
                                V1_sb[kb][:, h, :],
                                start=(kb == 0 and h == 0),
                                stop=(kb == qc * 4 + qbl),
                                skip_group_check=True,
                            )
                if p == 1:
                    pt_tiles.pop((j, 0), None)
                    pt_tiles.pop((j, 1), None)

            def emit_norm_qb(qc, qbl, tail=False):
                # tail mode: the exp stream is done, so the Activation engine
                # is free -- normalize there to dodge the DVE queue latency
                o = pso_tiles.pop((qc, qbl))
                qbg = qc * 4 + qbl
                zc = stg.tile(
                    [128, 4, 1], F32, tag="zc", bufs=4, name=f"zc{qbg}"
                )
                (nc.scalar.copy if tail else nc.vector.tensor_copy)(
                    zc, o[:, :, 64:65]
                )
                rc = stg.tile(
                    [128, 4, 1], F32, tag="rc", bufs=4, name=f"rc{qbg}"
                )
                nc.vector.reciprocal_approx_fast(out=rc, in_=zc)
                for h in range(4):
                    if tail:
                        nc.scalar.activation(
                            ATn_sb[:, qbg, h * 64 : (h + 1) * 64],
                            o[:, h, 0:64],
                            mybir.ActivationFunctionType.Copy,
                            scale=rc[:, h, 0:1],
                        )
                    else:
                        nc.vector.tensor_scalar_mul(
                            ATn_sb[:, qbg, h * 64 : (h + 1) * 64],
                            o[:, h, 0:64],
                            rc[:, h, 0:1],
                        )

            def emit_transpose(qc, pair=None):
                # f-major AT^T via the DMA xbar (off all compute engines)
                if pair is None:
                    nc.sync.dma_start_transpose(
                        out=ATt_sb[qc], in_=ATn_sb[:, qc * 4 : (qc + 1) * 4, :]
                    )
                else:
                    nc.sync.dma_start_transpose(
                        out=ATt_sb[qc][:, 2 * pair : 2 * pair + 2, :, :],
                        in_=ATn_sb[:, qc * 4 + 2 * pair : qc * 4 + 2 * pair + 2, :],
                    )

            ost_n = [0]
            dma_queue = []
            stg2_box = []

            def flush_dmas():
                while dma_queue:
                    ost, w, row, cols = dma_queue.pop(0)
                    dma0 = [nc.sync, nc.scalar][ost_n[0] % 2]
                    dma0.dma_start(
                        out=outT[row : row + 128, cols], in_=ost[:, 0, 0:w]
                    )
                    dma1 = [nc.scalar, nc.sync][ost_n[0] % 2]
                    dma1.dma_start(
                        out=outT[row + 128 : row + 256, cols],
                        in_=ost[:, 1, 0:w],
                    )

            def emit_proj(qc, jt2, qhalf=None):
                # qhalf None: full piece (q-cols 0..511 of the chunk);
                # 0/1: half piece over 2 q-blocks (tail-latency mode)
                psp = pss_pool.tile(
                    [128, 2, 512], F32, tag="s", name=f"pp{qc}_{jt2}_{qhalf}"
                )
                if qhalf is None:
                    qsl = slice(0, 4)
                    csl = slice(0, 512)
                else:
                    qsl = slice(2 * qhalf, 2 * qhalf + 2)
                    csl = slice(256 * qhalf, 256 * qhalf + 256)
                w = csl.stop - csl.start
                for sub in range(2):
                    dblk = 2 * jt2 + sub
                    for ft in range(2):
                        nc.tensor.matmul(
                            psp[:, sub, 0:w],
                            wp_sb[:, ft, dblk * 128 : (dblk + 1) * 128],
                            ATt_sb[qc][:, qsl, ft, :],
                            start=(ft == 0),
                            stop=(ft == 1),
                        )
                ost = stg2_box[0].tile(
                    [128, 2, 512], BF16, tag="ost", bufs=6,
                    name=f"ost{qc}{jt2}{qhalf}",
                )
                cols = slice(qc * 512 + csl.start, qc * 512 + csl.stop)
                # steady-state pieces stage both halves on DVE; tail
                # pieces stage half0 on DVE and half1 on Act (idle post-exp)
                # so the two copies run in parallel
                ceng = nc.vector.tensor_copy
                ceng1 = ceng
                ceng(ost[:, 0, 0:w], psp[:, 0, 0:w])
                ceng1(ost[:, 1, 0:w], psp[:, 1, 0:w])
                # defer the DMA issues one slot: by then the staging copies
                # have landed, so the DMA's SEQ-level wait is ~zero and never
                # blocks the exp stream (Act queue) behind it
                dma_queue.append(
                    (ost, w, 2 * jt2 * 128, cols)
                )
                ost_n[0] += 1

            with (
                tc.tile_pool(name="psO", bufs=4, space="PSUM") as pso_pool,
                tc.tile_pool(name="stage2", bufs=1) as stg2,
            ):
                stg2_box.append(stg2)
                proj_queue = []
                proj_cool = [0]
                S_ptr = [0]

                def alloc_O(qc):
                    for qbl in range(4):
                        pso_tiles[(qc, qbl)] = pso_pool.tile(
                            [128, 4, 65], F32, tag="o", name=f"o{qc}_{qbl}"
                        )

                while S_ptr[0] < NS and (pv_steps[S_ptr[0]], 1) in emitted:
                    S_ptr[0] += 1
                alloc_O(PV_ORDER[0])
                for jj in range(NS):
                    j = pv_steps[jj]
                    flush_dmas()
                    emit_tri(j)
                    if jj + 1 < NS:
                        emit_tri(pv_steps[jj + 1])
                    # proj before this slot's S-emissions: its psum slot (the
                    # shared "s" ring) then waits on an exp that is already
                    # done instead of one just emitted
                    if proj_queue and proj_cool[0] <= 0:
                        emit_proj(*proj_queue.pop(0))
                        proj_cool[0] = COOL
                    else:
                        proj_cool[0] -= 1
                    tgt = min(jj + DEPTH_B, NS - 1)
                    while S_ptr[0] <= tgt:
                        i = pv_steps[S_ptr[0]]
                        for p in range(2):
                            if (i, p) not in emitted:
                                emit_S(i, p)
                        S_ptr[0] += 1
                    qc, regs = steps[j]
                    emit_PV(j, 0)
                    emit_PV(j, 1)
                    # normalize each q-block at its stop step: spreads the
                    # norm work, frees O banks early (next chunk's PV never
                    # waits), and lets the tail chunk's proj start before the
                    # chunk's own last PV
                    tail = qc == LAST_QC
                    for kb in (r[0] for r in regs):
                        qbl = kb - 4 * qc
                        if qbl < 0:
                            continue
                        emit_norm_qb(qc, qbl, tail=tail)
                        if qbl == 1:
                            emit_transpose(qc, pair=0)
                            if tail:
                                for jt2 in range(4):
                                    proj_queue.append((qc, jt2, 0))
                        elif qbl == 3:
                            emit_transpose(qc, pair=1)
                            if tail:
                                while proj_queue:
                                    emit_proj(*proj_queue.pop(0))
                                for jt2 in range(4):
                                    emit_proj(qc, jt2, 1)
                            else:
                                for jt2 in range(4):
                                    proj_queue.append((qc, jt2, None))
                                nqc = PV_ORDER[PV_ORDER.index(qc) + 1]
                                alloc_O(nqc)
                while proj_queue:
                    emit_proj(*proj_queue.pop(0))
                    flush_dmas()
                flush_dmas()

    nc.finalize()
    return nc


_NC_CACHE = None


def _get_nc():
    global _NC_CACHE
    if _NC_CACHE is None:
        _NC_CACHE = build_nc()
    return _NC_CACHE


def make_in_maps(x, w_qkv, w_proj):
    import ml_dtypes

    bf16 = ml_dtypes.bfloat16
    x = np.asarray(x, dtype=np.float32).astype(bf16)
    w_qkv = np.asarray(w_qkv, dtype=np.float32).astype(bf16)
    w_proj = np.asarray(w_proj, dtype=np.float32).astype(bf16)
    tri = np.ascontiguousarray(
        np.broadcast_to(
            np.triu(np.ones((128, 128), dtype=np.float32))[:, None, :],
            (128, 2, 128),
        )
    ).astype(bf16)
    in_maps = []
    for c in range(N_CORES):
        b, g = divmod(c, 4)
        cs = 256 * g
        in_maps.append(
            {
                "xT": np.ascontiguousarray(x[b].T),
                "wqk": np.ascontiguousarray(
                    np.concatenate(
                        [w_qkv[:, cs : cs + 256], w_qkv[:, D + cs : D + cs + 256]],
                        axis=1,
                    )
                ),
                "wv": np.ascontiguousarray(w_qkv[:, 2 * D + cs : 2 * D + cs + 256]),
                "wp": np.ascontiguousarray(w_proj[cs : cs + 256, :]),
                "tric": tri,
            }
        )
    return in_maps


def assemble(results):
    out = np.empty((B, T, D), dtype=np.float32)
    for b in range(B):
        acc = results[4 * b]["outT"].astype(np.float32)
        for g in range(1, 4):
            acc = acc + results[4 * b + g]["outT"]
        out[b] = acc.T
    return out


def kernel(x, w_qkv, w_proj, trace=False):
    nc = _get_nc()
    in_maps = make_in_maps(x, w_qkv, w_proj)
    res = bass_utils.run_bass_kernel_spmd(
        nc, in_maps, core_ids=list(range(N_CORES)), trace=trace
    )
    out = assemble(res.results)
    if trace:
        kernel.last_exec_time_ns = res.exec_time_ns
        kernel.last_result = res
    return out


# revision 37
# speedup vs baseline: 1.0077x; 1.0077x over previous
"""Causal multi-head attention (B=2, T=2048, D=1024, NH=16, HD=64) on 8 trn2
NeuronCores.

Sharding: data-parallel over batch (2) x tensor-parallel over head groups (4),
Megatron-style. Core c handles batch c//4, heads 4*(c%4)..4*(c%4)+3; the host
sums the 4 partial output projections per batch.

Layout/schedule (vs the O^T baseline, 156.3us -> 131.8us):
- PV runs in NATURAL layout: O[q,hd+1] accumulates with the P^T block as the
  STATIONARY operand (ldweights is engine-free) and V (with a ones column for
  the softmax denominator Z) as the moving operand, so each PV matmul has 128
  output partitions and only 65 moving columns -- half the PE cycles of the
  O^T form (65/128-partition waste eliminated). PSUM start-zeroing is
  bank-granular, so exactly one matmul per O tile carries start=True; the
  other heads' first writes land on its bank-wide pending-zero.
- Normalization becomes per-partition scalars: 1/Z via
  reciprocal_approx_fast on [128,4,1] tiles, applied with tensor_scalar_mul,
  emitted at each q-block's PV *stop* step so O banks recycle early.
- The f-major layout the projection needs is produced by dma_start_transpose
  (XBAR 16x128 tiles, 14ns each on the DMA engines -- off every compute
  engine). Projection then consumes AT^T blocks with wp stationary.
- Activation engine runs the exp stream (the attention-phase co-critical
  resource); psum->sbuf staging goes to DVE, except the tail where Act is
  free. Output DMAs are issued one slot late so their SEQ-level waits never
  block the exp stream on the Act queue.
- qkv is overlapped with early attention: 8 concurrent kt-outer psum groups
  (4 psa + the idle psS slots) track the per-kt x DMA arrivals, then S/exp
  steps interleave with the remaining qkv/V psum groups so Act starts
  exp'ing ~11us in. PVs are deferred until the qkv psum pools close (PSUM
  budget: 8 banks exactly in each phase). The sim's engines execute past
  blocked instructions, so phase B emits all remaining S steps up front and
  lets the psS/pt/O rings self-pace.
- S/exp emission and PV/norm/proj processing both end on a LONG chunk
  (PV_ORDER [1,3,0,2]) whose spread-out norms + early proj halves hide the
  tail normalize/transpose chains.
"""

import sys

if "/opt/trn_rl_repo" not in sys.path:
    sys.path.insert(0, "/opt/trn_rl_repo")

import numpy as np
import concourse.mybir as mybir
from concourse import bacc
from concourse.tile import TileContext
from concourse import bass_utils

B, T, D = 2, 2048, 1024
NH, HD = 16, 64
HL = 4  # heads per core
N_CORES = 8

KT = D // 128  # 8 contraction tiles over model dim
TT = T // 128  # 16 t-blocks of 128

F32 = mybir.dt.float32
BF16 = mybir.dt.bfloat16

QC_ORDER = [1, 3, 2, 0]
DEPTH_B = 4  # phase-B S-emission lookahead (steps) over PV consumption
COOL = 1  # slots between woven proj pieces
OVERLAP_CAP = 52  # max S items emitted during the qkv overlap (pt-pool bound)
OV_SPG = 2  # S items interleaved per qkv/V psum group


def build_nc():
    nc = bacc.Bacc()
    xT = nc.dram_tensor("xT", [D, T], BF16, kind="ExternalInput")
    wqk = nc.dram_tensor("wqk", [D, 512], BF16, kind="ExternalInput")
    wv = nc.dram_tensor("wv", [D, 256], BF16, kind="ExternalInput")
    wp = nc.dram_tensor("wp", [256, D], BF16, kind="ExternalInput")
    tric = nc.dram_tensor("tric", [128, 2, 128], BF16, kind="ExternalInput")
    outT = nc.dram_tensor("outT", [D, T], BF16, kind="ExternalOutput")

    # attention step list: (qc, [(kb, rlo, rhi, slo), ...]); the last two
    # diagonal blocks of each q-chunk are packed into one psum tile / one exp
    steps = []
    for qc in QC_ORDER:
        nkb = 4 * qc + 4
        for kb in range(nkb - 2):
            lo = max(128 * (kb - 4 * qc), 0)
            steps.append((qc, [(kb, lo, 512, lo)]))
        steps.append(
            (qc, [(nkb - 2, 256, 512, 256), (nkb - 1, 128, 256, 384)])
        )
    NS = len(steps)
    # PV/normalize/proj processing order AND phase-B S-emission order: both
    # end on a LONG chunk (its norm-at-stop spreading + early proj halves
    # hide the tail chains). The overlap S order stays QC_ORDER.
    PV_ORDER = [1, 3, 0, 2]
    pv_steps = [
        i for qc in PV_ORDER for i, (q, _) in enumerate(steps) if q == qc
    ]
    LAST_QC = PV_ORDER[-1]

    with TileContext(nc) as tc:
        with (
            tc.tile_pool(name="persist", bufs=1) as pers,
            tc.tile_pool(name="ptile", bufs=52) as ppool,
            tc.tile_pool(name="stage", bufs=1) as stg,
            tc.tile_pool(name="psS", bufs=2, space="PSUM") as pss_pool,
        ):
            qkT_sb = [
                pers.tile([128, T], BF16, tag=f"qkT{mt}", name=f"qkT{mt}")
                for mt in range(4)
            ]
            tri_sb = pers.tile([128, 2, 128], BF16, tag="tri", name="tri")
            V1_sb = [
                pers.tile([128, HL, 65], BF16, tag=f"V1_{tt}", name=f"V1_{tt}")
                for tt in range(TT)
            ]
            # AT natural [q-part, (qb 16, f 256)]; f = (head, hd) per core
            ATn_sb = pers.tile([128, TT, 256], BF16, tag="ATn", name="ATn")
            # AT^T per chunk: blocks (qb-local, f-half) of [128 f, 128 q]
            ATt_sb = [
                pers.tile([128, 4, 2, 128], BF16, tag=f"ATt{qc}", name=f"ATt{qc}")
                for qc in range(4)
            ]
            wp_sb = pers.tile([128, 2, D], BF16, tag="wp", name="wp")

            # ---------------- S/exp emission machinery ----------------
            emitted = set()
            tri_done = set()
            qk_copied = set()
            pt_tiles = {}

            def step_ready(i, p):
                qc, regs = steps[i]
                if (p, qc) not in qk_copied:
                    return False
                maxkb = max(r[0] for r in regs)
                return all(
                    (2 + p, g) in qk_copied for g in range(maxkb // 4 + 1)
                )

            def emit_S(i, p):
                qc, regs = steps[i]
                tlo = min(r[1] for r in regs)
                thi = max(r[2] for r in regs)
                qT = qkT_sb[p]
                kT = qkT_sb[2 + p]
                psS = pss_pool.tile([128, 2, 512], F32, tag="s", name=f"s{p}_{i}")
                pt = ppool.tile([128, 2, 512], BF16, tag="pt", name=f"pt{p}_{i}")
                for kb, rlo, rhi, slo in regs:
                    for hslot in range(2):
                        nc.tensor.matmul(
                            psS[:, hslot, rlo:rhi],
                            kT[
                                64 * hslot : 64 * hslot + 64,
                                kb * 128 : (kb + 1) * 128,
                            ],
                            qT[
                                64 * hslot : 64 * hslot + 64,
                                qc * 512 + slo : qc * 512 + slo + (rhi - rlo),
                            ],
                            start=True,
                            stop=True,
                        )
                nc.scalar.activation(
                    pt[:, :, tlo:thi],
                    psS[:, :, tlo:thi],
                    mybir.ActivationFunctionType.Exp,
                    scale=0.125,
                )
                pt_tiles[(i, p)] = pt
                emitted.add((i, p))

            def emit_tri(i):
                # causal zeroing of the diagonal 128-band; deferred until just
                # before the consuming PV so the DVE never waits on a lagging
                # exp (the exp finished long ago by PV time)
                if i in tri_done or i >= NS:
                    return
                qc, regs = steps[i]
                for p in range(2):
                    pt = pt_tiles[(i, p)]
                    for kb, rlo, rhi, slo in regs:
                        if 128 * kb >= slo + qc * 512:
                            nc.vector.tensor_mul(
                                pt[:, :, rlo : rlo + 128],
                                pt[:, :, rlo : rlo + 128],
                                tri_sb[:, :, 0:128],
                            )
                tri_done.add(i)

            item_queue = [(i, p) for i in range(NS) for p in range(2)]
            n_ov = [0]

            def ov_emit(max_n):
                n = 0
                k = 0
                while (
                    n < max_n
                    and n_ov[0] < OVERLAP_CAP
                    and k < len(item_queue)
                ):
                    it = item_queue[k]
                    if it in emitted:
                        item_queue.pop(k)
                        continue
                    if step_ready(*it):
                        emit_S(*it)
                        item_queue.pop(k)
                        n += 1
                        n_ov[0] += 1
                    else:
                        k += 1
                return n

            # ---------------- phase A: qkv + V, overlapped ----------------
            with tc.tile_pool(name="qkv_in", bufs=1) as qin:
                xT_all = qin.tile([128, KT, T], BF16, tag="xT", name="xT_all")
                wqk_all = qin.tile(
                    [128, KT, 512], BF16, tag="wqk", name="wqk_all"
                )
                wv_all = qin.tile([128, KT, 256], BF16, tag="wv", name="wv_all")

                # load order tuned for the A1 wave: wqk kt0, x first halves
                # (wave1 reads only T-cols 0..1023), wqk rest, x second
                # halves, then V/proj weights. x halves arrive every ~0.7us
                # so the kt-outer wave1 matmuls start ~2.5us in and stay fed.
                # weights-only on the sync queue; all x plus the late
                # weights on the scalar queue -- the HWDGE alternates queues,
                # so this keeps the x first-half stream dense (the kt-outer
                # wave1 is paced by it) and pushes second halves strictly
                # after it
                nc.scalar.dma_start(
                    out=xT_all[:, 0, 0:512], in_=xT[0:128, 0:512]
                )
                nc.sync.dma_start(out=wqk_all[:, 0, :], in_=wqk[0:128, :])
                nc.scalar.dma_start(
                    out=xT_all[:, 0, 512:1024], in_=xT[0:128, 512:1024]
                )
                nc.sync.dma_start(
                    out=wqk_all[:, 1:3, :],
                    in_=wqk[128:384, :].rearrange("(k p) c -> p k c", k=2),
                )
                nc.scalar.dma_start(
                    out=xT_all[:, 1, 0:1024], in_=xT[128:256, 0:1024]
                )
                nc.sync.dma_start(
                    out=wqk_all[:, 3:5, :],
                    in_=wqk[384:640, :].rearrange("(k p) c -> p k c", k=2),
                )
                nc.scalar.dma_start(
                    out=xT_all[:, 2, 0:1024], in_=xT[256:384, 0:1024]
                )
                nc.sync.dma_start(
                    out=wqk_all[:, 5:KT, :],
                    in_=wqk[640:D, :].rearrange("(k p) c -> p k c", k=3),
                )
                for kt in range(3, KT):
                    nc.scalar.dma_start(
                        out=xT_all[:, kt, 0:1024],
                        in_=xT[kt * 128 : (kt + 1) * 128, 0:1024],
                    )
                for kt in range(KT):
                    eng = [nc.sync, nc.scalar][kt % 2]
                    eng.dma_start(
                        out=xT_all[:, kt, 1024:2048],
                        in_=xT[kt * 128 : (kt + 1) * 128, 1024:2048],
                    )
                nc.scalar.dma_start(
                    out=wv_all,
                    in_=wv[:, :].rearrange("(k p) c -> p k c", k=KT),
                )
                for ft in range(2):
                    nc.scalar.dma_start(
                        out=wp_sb[:, ft, :], in_=wp[ft * 128 : (ft + 1) * 128, :]
                    )
                nc.scalar.dma_start(out=tri_sb, in_=tric[:, :, :])
                for tt in range(TT):
                    nc.gpsimd.memset(V1_sb[tt][:, :, 64:65], 1.0)

                with tc.tile_pool(name="psA1", bufs=4, space="PSUM") as psa1:
                    # wave1: ALL m-tiles x T-cols 0..1023, kt-outer so the
                    # matmul stream tracks the per-kt x DMA arrivals. 8
                    # concurrent psum groups: 4 from psa1 plus the 4 halves
                    # of the 2 psS ring slots (idle until the first S step),
                    # so the PE is ~fully busy during the paced x load.
                    wave1 = [(0, 0), (2, 0), (0, 1), (2, 1)]
                    wave1b = [(1, 0), (3, 0), (1, 1), (3, 1)]
                    pstiles = {}
                    for mt, g in wave1:
                        pstiles[(mt, g)] = psa1.tile(
                            [128, 512], F32, tag="qkps", name=f"qk{mt}{g}"
                        )
                    sslots = [
                        pss_pool.tile([128, 2, 512], F32, tag="s", name=f"w1b{i}")
                        for i in range(2)
                    ]
                    for i, (mt, g) in enumerate(wave1b):
                        pstiles[(mt, g)] = sslots[i // 2][:, i % 2, :]
                    for kt in range(KT):
                        for mt, g in wave1 + wave1b:
                            nc.tensor.matmul(
                                pstiles[(mt, g)],
                                wqk_all[:, kt, mt * 128 : (mt + 1) * 128],
                                xT_all[:, kt, g * 512 : (g + 1) * 512],
                                start=(kt == 0),
                                stop=(kt == KT - 1),
                            )
                    for i, (mt, g) in enumerate(wave1 + wave1b):
                        # split copies across Act (idle before the first exp)
                        # and DVE
                        eng = nc.scalar.copy if i % 2 == 0 else (
                            lambda o, s: nc.vector.tensor_copy(o, s)
                        )
                        eng(
                            qkT_sb[mt][:, g * 512 : (g + 1) * 512],
                            pstiles[(mt, g)],
                        )
                        qk_copied.add((mt, g))
                    ov_emit(6)
                    # remaining qk groups, kt-inner, ordered to unlock S
                    # items (qc0 p1 first, then qc3 p0, ...) as fast as Act
                    # can consume them
                    rest = [
                        (0, 3), (2, 2), (2, 3), (3, 2),
                        (3, 3), (1, 3), (0, 2), (1, 2),
                    ]
                    for gi, (mt, g) in enumerate(rest):
                        ps = psa1.tile(
                            [128, 512], F32, tag="qkps", name=f"qk{mt}{g}"
                        )
                        for kt in range(KT):
                            nc.tensor.matmul(
                                ps,
                                wqk_all[:, kt, mt * 128 : (mt + 1) * 128],
                                xT_all[:, kt, g * 512 : (g + 1) * 512],
                                start=(kt == 0),
                                stop=(kt == KT - 1),
                            )
                        nc.vector.tensor_copy(
                            qkT_sb[mt][:, g * 512 : (g + 1) * 512], ps
                        )
                        qk_copied.add((mt, g))
                        ov_emit(1 if gi < 4 else OV_SPG)

                with tc.tile_pool(name="psB", bufs=2, space="PSUM") as psb:
                    for tt in range(TT):
                        psv = psb.tile([128, 256], F32, tag="v", name=f"v{tt}")
                        for kt in range(KT):
                            nc.tensor.matmul(
                                psv,
                                xT_all[:, kt, tt * 128 : (tt + 1) * 128],
                                wv_all[:, kt, :],
                                start=(kt == 0),
                                stop=(kt == KT - 1),
                            )
                        nc.vector.tensor_copy(V1_sb[tt][:, :, 0:64], psv)
                        ov_emit(OV_SPG)

            # ---------------- phase B: PV / normalize / proj ----------------
            pso_tiles = {}

            def emit_PV(j, p):
                qc, regs = steps[j]
                pt = pt_tiles[(j, p)]
                # psum start-zeroing is bank-granular: exactly ONE matmul per
                # O tile (the first to touch the bank: kb==0, h==0) carries
                # start=True. Its bank-wide pending-zero mark makes the other
                # heads' first writes land on zeros, and later accumulates
                # add normally -- no separate memset needed.
                for kb, rlo, rhi, slo in regs:
                    for hslot in range(2):
                        h = 2 * p + hslot
                        for qbl in range(slo // 128, (slo + rhi - rlo) // 128):
                            off = qbl * 128 - slo
                            nc.tensor.matmul(
                                pso_tiles[(qc, qbl)][:, h, :],
                                pt[:, hslot, rlo + off : rlo + off + 128],
                                V1_sb[kb][:, h, :],
                                start=(kb == 0 and h == 0),
                                stop=(kb == qc * 4 + qbl),
                                skip_group_check=True,
                            )
                if p == 1:
                    pt_tiles.pop((j, 0), None)
                    pt_tiles.pop((j, 1), None)

            def emit_norm_qb(qc, qbl, tail=False):
                # tail mode: the exp stream is done, so the Activation engine
                # is free -- normalize there to dodge the DVE queue latency
                o = pso_tiles.pop((qc, qbl))
                qbg = qc * 4 + qbl
                zc = stg.tile(
                    [128, 4, 1], F32, tag="zc", bufs=4, name=f"zc{qbg}"
                )
                (nc.scalar.copy if tail else nc.vector.tensor_copy)(
                    zc, o[:, :, 64:65]
                )
                rc = stg.tile(
                    [128, 4, 1], F32, tag="rc", bufs=4, name=f"rc{qbg}"
                )
                nc.vector.reciprocal_approx_fast(out=rc, in_=zc)
                for h in range(4):
                    if tail:
                        nc.scalar.activation(
                            ATn_sb[:, qbg, h * 64 : (h + 1) * 64],
                            o[:, h, 0:64],
                            mybir.ActivationFunctionType.Copy,
                            scale=rc[:, h, 0:1],
                        )
                    else:
                        nc.vector.tensor_scalar_mul(
                            ATn_sb[:, qbg, h * 64 : (h + 1) * 64],
                            o[:, h, 0:64],
                            rc[:, h, 0:1],
                        )

            def emit_transpose(qc, pair=None):
                # f-major AT^T via the DMA xbar (off all compute engines)
                if pair is None:
                    nc.sync.dma_start_transpose(
                        out=ATt_sb[qc], in_=ATn_sb[:, qc * 4 : (qc + 1) * 4, :]
                    )
                else:
                    nc.sync.dma_start_transpose(
                        out=ATt_sb[qc][:, 2 * pair : 2 * pair + 2, :, :],
                        in_=ATn_sb[:, qc * 4 + 2 * pair : qc * 4 + 2 * pair + 2, :],
                    )

            ost_n = [0]
            dma_queue = []
            stg2_box = []

            def flush_dmas():
                while dma_queue:
                    ost, w, row, cols = dma_queue.pop(0)
                    dma0 = [nc.sync, nc.scalar][ost_n[0] % 2]
                    dma0.dma_start(
                        out=outT[row : row + 128, cols], in_=ost[:, 0, 0:w]
                    )
                    dma1 = [nc.scalar, nc.sync][ost_n[0] % 2]
                    dma1.dma_start(
                        out=outT[row + 128 : row + 256, cols],
                        in_=ost[:, 1, 0:w],
                    )

            def emit_proj(qc, jt2, qhalf=None):
                # qhalf None: full piece (q-cols 0..511 of the chunk);
                # 0/1: half piece over 2 q-blocks (tail-latency mode)
                psp = pss_pool.tile(
                    [128, 2, 512], F32, tag="s", name=f"pp{qc}_{jt2}_{qhalf}"
                )
                if qhalf is None:
                    qsl = slice(0, 4)
                    csl = slice(0, 512)
                else:
                    qsl = slice(2 * qhalf, 2 * qhalf + 2)
                    csl = slice(256 * qhalf, 256 * qhalf + 256)
                w = csl.stop - csl.start
                for sub in range(2):
                    dblk = 2 * jt2 + sub
                    for ft in range(2):
                        nc.tensor.matmul(
                            psp[:, sub, 0:w],
                            wp_sb[:, ft, dblk * 128 : (dblk + 1) * 128],
                            ATt_sb[qc][:, qsl, ft, :],
                            start=(ft == 0),
                            stop=(ft == 1),
                        )
                ost = stg2_box[0].tile(
                    [128, 2, 512], BF16, tag="ost", bufs=6,
                    name=f"ost{qc}{jt2}{qhalf}",
                )
                cols = slice(qc * 512 + csl.start, qc * 512 + csl.stop)
                # steady-state pieces stage both halves on DVE; tail
                # pieces stage half0 on DVE and half1 on Act (idle post-exp)
                # so the two copies run in parallel
                ceng = nc.vector.tensor_copy
                ceng1 = ceng
                ceng(ost[:, 0, 0:w], psp[:, 0, 0:w])
                ceng1(ost[:, 1, 0:w], psp[:, 1, 0:w])
                # defer the DMA issues one slot: by then the staging copies
                # have landed, so the DMA's SEQ-level wait is ~zero and never
                # blocks the exp stream (Act queue) behind it
                dma_queue.append(
                    (ost, w, 2 * jt2 * 128, cols)
                )
                ost_n[0] += 1

            with (
                tc.tile_pool(name="psO", bufs=4, space="PSUM") as pso_pool,
                tc.tile_pool(name="stage2", bufs=1) as stg2,
            ):
                stg2_box.append(stg2)
                proj_queue = []
                proj_cool = [0]
                S_ptr = [0]

                def alloc_O(qc):
                    for qbl in range(4):
                        pso_tiles[(qc, qbl)] = pso_pool.tile(
                            [128, 4, 65], F32, tag="o", name=f"o{qc}_{qbl}"
                        )

                while S_ptr[0] < NS and (pv_steps[S_ptr[0]], 1) in emitted:
                    S_ptr[0] += 1
                alloc_O(PV_ORDER[0])
                for jj in range(NS):
                    j = pv_steps[jj]
                    flush_dmas()
                    emit_tri(j)
                    if jj + 1 < NS:
                        emit_tri(pv_steps[jj + 1])
                    # proj before this slot's S-emissions: its psum slot (the
                    # shared "s" ring) then waits on an exp that is already
                    # done instead of one just emitted
                    if proj_queue and proj_cool[0] <= 0:
                        emit_proj(*proj_queue.pop(0))
                        proj_cool[0] = COOL
                    else:
                        proj_cool[0] -= 1
                    tgt = min(jj + DEPTH_B, NS - 1)
                    while S_ptr[0] <= tgt:
                        i = pv_steps[S_ptr[0]]
                        for p in range(2):
                            if (i, p) not in emitted:
                                emit_S(i, p)
                        S_ptr[0] += 1
                    qc, regs = steps[j]
                    emit_PV(j, 0)
                    emit_PV(j, 1)
                    # normalize each q-block at its stop step: spreads the
                    # norm work, frees O banks early (next chunk's PV never
                    # waits), and lets the tail chunk's proj start before the
                    # chunk's own last PV
                    tail = qc == LAST_QC
                    for kb in (r[0] for r in regs):
                        qbl = kb - 4 * qc
                        if qbl < 0:
                            continue
                        emit_norm_qb(qc, qbl, tail=tail)
                        if qbl == 1:
                            emit_transpose(qc, pair=0)
                            if tail:
                                for jt2 in range(4):
                                    proj_queue.append((qc, jt2, 0))
                        elif qbl == 3:
                            emit_transpose(qc, pair=1)
                            if tail:
                                while proj_queue:
                                    emit_proj(*proj_queue.pop(0))
                                for jt2 in range(4):
                                    emit_proj(qc, jt2, 1)
                            else:
                                for jt2 in range(4):
                                    proj_queue.append((qc, jt2, None))
                                nqc = PV_ORDER[PV_ORDER.index(qc) + 1]
                                alloc_O(nqc)
                while proj_queue:
                    emit_proj(*proj_queue.pop(0))
                    flush_dmas()
                flush_dmas()

    nc.finalize()
    return nc


_NC_CACHE = None


def _get_nc():
    global _NC_CACHE
    if _NC_CACHE is None:
        _NC_CACHE = build_nc()
    return _NC_CACHE


def make_in_maps(x, w_qkv, w_proj):
    import ml_dtypes

    bf16 = ml_dtypes.bfloat16
    x = np.asarray(x, dtype=np.float32).astype(bf16)
    w_qkv = np.asarray(w_qkv, dtype=np.float32).astype(bf16)
    w_proj = np.asarray(w_proj, dtype=np.float32).astype(bf16)
    tri = np.ascontiguousarray(
        np.broadcast_to(
            np.triu(np.ones((128, 128), dtype=np.float32))[:, None, :],
            (128, 2, 128),
        )
    ).astype(bf16)
    in_maps = []
    for c in range(N_CORES):
        b, g = divmod(c, 4)
        cs = 256 * g
        in_maps.append(
            {
                "xT": np.ascontiguousarray(x[b].T),
                "wqk": np.ascontiguousarray(
                    np.concatenate(
                        [w_qkv[:, cs : cs + 256], w_qkv[:, D + cs : D + cs + 256]],
                        axis=1,
                    )
                ),
                "wv": np.ascontiguousarray(w_qkv[:, 2 * D + cs : 2 * D + cs + 256]),
                "wp": np.ascontiguousarray(w_proj[cs : cs + 256, :]),
                "tric": tri,
            }
        )
    return in_maps


def assemble(results):
    out = np.empty((B, T, D), dtype=np.float32)
    for b in range(B):
        acc = results[4 * b]["outT"].astype(np.float32)
        for g in range(1, 4):
            acc = acc + results[4 * b + g]["outT"]
        out[b] = acc.T
    return out


def kernel(x, w_qkv, w_proj, trace=False):
    nc = _get_nc()
    in_maps = make_in_maps(x, w_qkv, w_proj)
    res = bass_utils.run_bass_kernel_spmd(
        nc, in_maps, core_ids=list(range(N_CORES)), trace=trace
    )
    out = assemble(res.results)
    if trace:
        kernel.last_exec_time_ns = res.exec_time_ns
        kernel.last_result = res
    return out


# revision 40
# speedup vs baseline: 1.0140x; 1.0062x over previous
"""Causal multi-head attention (B=2, T=2048, D=1024, NH=16, HD=64) on 8 trn2
NeuronCores.

Sharding: data-parallel over batch (2) x tensor-parallel over head groups (4),
Megatron-style. Core c handles batch c//4, heads 4*(c%4)..4*(c%4)+3; the host
sums the 4 partial output projections per batch.

Layout/schedule (vs the O^T baseline, 156.3us -> 129.6us):
- PV runs in NATURAL layout: O[q,hd+1] accumulates with the P^T block as the
  STATIONARY operand (ldweights is engine-free) and V (with a ones column for
  the softmax denominator Z) as the moving operand, so each PV matmul has 128
  output partitions and only 65 moving columns -- half the PE cycles of the
  O^T form (65/128-partition waste eliminated). PSUM start-zeroing is
  bank-granular, so exactly one matmul per O tile carries start=True; the
  other heads' first writes land on its bank-wide pending-zero.
- Normalization becomes per-partition scalars: 1/Z via
  reciprocal_approx_fast on [128,4,1] tiles, applied with tensor_scalar_mul,
  emitted at each q-block's PV *stop* step so O banks recycle early.
- The f-major layout the projection needs is produced by dma_start_transpose
  (XBAR 16x128 tiles, 14ns each on the DMA engines -- off every compute
  engine). Projection then consumes AT^T blocks with wp stationary.
- Activation engine runs the exp stream (the attention-phase co-critical
  resource); psum->sbuf staging goes to DVE, except the tail where Act is
  free. Output DMAs are issued one slot late so their SEQ-level waits never
  block the exp stream on the Act queue.
- qkv is overlapped with early attention: 8 concurrent kt-outer psum groups
  (4 psa + the idle psS slots) track the per-kt x DMA arrivals, then S/exp
  steps interleave with the remaining qkv/V psum groups so Act starts
  exp'ing ~11us in. PVs are deferred until the qkv psum pools close (PSUM
  budget: 8 banks exactly in each phase). The sim's engines execute past
  blocked instructions, so phase B emits all remaining S steps up front and
  lets the psS/pt/O rings self-pace.
- S/exp emission and PV/norm/proj processing both end on a LONG chunk
  (PV_ORDER [1,3,0,2]) whose spread-out norms + early proj halves hide the
  tail normalize/transpose chains.
"""

import sys

if "/opt/trn_rl_repo" not in sys.path:
    sys.path.insert(0, "/opt/trn_rl_repo")

import numpy as np
import concourse.mybir as mybir
from concourse import bacc
from concourse.tile import TileContext
from concourse import bass_utils

B, T, D = 2, 2048, 1024
NH, HD = 16, 64
HL = 4  # heads per core
N_CORES = 8

KT = D // 128  # 8 contraction tiles over model dim
TT = T // 128  # 16 t-blocks of 128

F32 = mybir.dt.float32
BF16 = mybir.dt.bfloat16

QC_ORDER = [1, 3, 2, 0]
DEPTH_B = 4  # phase-B S-emission lookahead (steps) over PV consumption
COOL = 1  # slots between woven proj pieces
OVERLAP_CAP = 52  # max S items emitted during the qkv overlap (pt-pool bound)
OV_SPG = 2  # S items interleaved per qkv/V psum group


def build_nc():
    nc = bacc.Bacc()
    xT = nc.dram_tensor("xT", [D, T], BF16, kind="ExternalInput")
    wqk = nc.dram_tensor("wqk", [D, 512], BF16, kind="ExternalInput")
    wv = nc.dram_tensor("wv", [D, 256], BF16, kind="ExternalInput")
    wp = nc.dram_tensor("wp", [256, D], BF16, kind="ExternalInput")
    tric = nc.dram_tensor("tric", [128, 2, 128], BF16, kind="ExternalInput")
    outT = nc.dram_tensor("outT", [D, T], BF16, kind="ExternalOutput")

    # attention step list: (qc, [(kb, rlo, rhi, slo), ...]); the last two
    # diagonal blocks of each q-chunk are packed into one psum tile / one exp
    steps = []
    for qc in QC_ORDER:
        nkb = 4 * qc + 4
        for kb in range(nkb - 2):
            lo = max(128 * (kb - 4 * qc), 0)
            steps.append((qc, [(kb, lo, 512, lo)]))
        steps.append(
            (qc, [(nkb - 2, 256, 512, 256), (nkb - 1, 128, 256, 384)])
        )
    NS = len(steps)
    # PV/normalize/proj processing order AND phase-B S-emission order: both
    # end on a LONG chunk (its norm-at-stop spreading + early proj halves
    # hide the tail chains). The overlap S order stays QC_ORDER.
    PV_ORDER = [1, 3, 0, 2]
    pv_steps = [
        i for qc in PV_ORDER for i, (q, _) in enumerate(steps) if q == qc
    ]
    LAST_QC = PV_ORDER[-1]

    with TileContext(nc) as tc:
        with (
            tc.tile_pool(name="persist", bufs=1) as pers,
            tc.tile_pool(name="ptile", bufs=52) as ppool,
            tc.tile_pool(name="stage", bufs=1) as stg,
            tc.tile_pool(name="psS", bufs=2, space="PSUM") as pss_pool,
        ):
            qkT_sb = [
                pers.tile([128, T], BF16, tag=f"qkT{mt}", name=f"qkT{mt}")
                for mt in range(4)
            ]
            tri_sb = pers.tile([128, 2, 128], BF16, tag="tri", name="tri")
            V1_sb = [
                pers.tile([128, HL, 65], BF16, tag=f"V1_{tt}", name=f"V1_{tt}")
                for tt in range(TT)
            ]
            # AT natural [q-part, (qb 16, f 256)]; f = (head, hd) per core
            ATn_sb = pers.tile([128, TT, 256], BF16, tag="ATn", name="ATn")
            # AT^T per chunk: blocks (qb-local, f-half) of [128 f, 128 q]
            ATt_sb = [
                pers.tile([128, 4, 2, 128], BF16, tag=f"ATt{qc}", name=f"ATt{qc}")
                for qc in range(4)
            ]
            wp_sb = pers.tile([128, 2, D], BF16, tag="wp", name="wp")

            # ---------------- S/exp emission machinery ----------------
            emitted = set()
            tri_done = set()
            qk_copied = set()
            pt_tiles = {}

            def step_ready(i, p):
                qc, regs = steps[i]
                if (p, qc) not in qk_copied:
                    return False
                maxkb = max(r[0] for r in regs)
                return all(
                    (2 + p, g) in qk_copied for g in range(maxkb // 4 + 1)
                )

            def emit_S(i, p):
                qc, regs = steps[i]
                tlo = min(r[1] for r in regs)
                thi = max(r[2] for r in regs)
                qT = qkT_sb[p]
                kT = qkT_sb[2 + p]
                psS = pss_pool.tile([128, 2, 512], F32, tag="s", name=f"s{p}_{i}")
                pt = ppool.tile([128, 2, 512], BF16, tag="pt", name=f"pt{p}_{i}")
                for kb, rlo, rhi, slo in regs:
                    for hslot in range(2):
                        nc.tensor.matmul(
                            psS[:, hslot, rlo:rhi],
                            kT[
                                64 * hslot : 64 * hslot + 64,
                                kb * 128 : (kb + 1) * 128,
                            ],
                            qT[
                                64 * hslot : 64 * hslot + 64,
                                qc * 512 + slo : qc * 512 + slo + (rhi - rlo),
                            ],
                            start=True,
                            stop=True,
                        )
                nc.scalar.activation(
                    pt[:, :, tlo:thi],
                    psS[:, :, tlo:thi],
                    mybir.ActivationFunctionType.Exp,
                    scale=0.125,
                )
                pt_tiles[(i, p)] = pt
                emitted.add((i, p))

            def emit_tri(i):
                # causal zeroing of the diagonal 128-band; deferred until just
                # before the consuming PV so the DVE never waits on a lagging
                # exp (the exp finished long ago by PV time)
                if i in tri_done or i >= NS:
                    return
                qc, regs = steps[i]
                for p in range(2):
                    pt = pt_tiles[(i, p)]
                    for kb, rlo, rhi, slo in regs:
                        if 128 * kb >= slo + qc * 512:
                            nc.vector.tensor_mul(
                                pt[:, :, rlo : rlo + 128],
                                pt[:, :, rlo : rlo + 128],
                                tri_sb[:, :, 0:128],
                            )
                tri_done.add(i)

            item_queue = [(i, p) for i in range(NS) for p in range(2)]
            n_ov = [0]

            def ov_emit(max_n):
                n = 0
                k = 0
                while (
                    n < max_n
                    and n_ov[0] < OVERLAP_CAP
                    and k < len(item_queue)
                ):
                    it = item_queue[k]
                    if it in emitted:
                        item_queue.pop(k)
                        continue
                    if step_ready(*it):
                        emit_S(*it)
                        item_queue.pop(k)
                        n += 1
                        n_ov[0] += 1
                    else:
                        k += 1
                return n

            # ---------------- phase A: qkv + V, overlapped ----------------
            with tc.tile_pool(name="qkv_in", bufs=1) as qin:
                xT_all = qin.tile([128, KT, T], BF16, tag="xT", name="xT_all")
                wqk_all = qin.tile(
                    [128, KT, 512], BF16, tag="wqk", name="wqk_all"
                )
                wv_all = qin.tile([128, KT, 256], BF16, tag="wv", name="wv_all")

                # load order tuned for the A1 wave: wqk kt0, x first halves
                # (wave1 reads only T-cols 0..1023), wqk rest, x second
                # halves, then V/proj weights. x halves arrive every ~0.7us
                # so the kt-outer wave1 matmuls start ~2.5us in and stay fed.
                # weights-only on the sync queue; all x plus the late
                # weights on the scalar queue -- the HWDGE alternates queues,
                # so this keeps the x first-half stream dense (the kt-outer
                # wave1 is paced by it) and pushes second halves strictly
                # after it
                nc.scalar.dma_start(
                    out=xT_all[:, 0, 0:512], in_=xT[0:128, 0:512]
                )
                nc.sync.dma_start(out=wqk_all[:, 0, :], in_=wqk[0:128, :])
                nc.scalar.dma_start(
                    out=xT_all[:, 0, 512:1024], in_=xT[0:128, 512:1024]
                )
                nc.sync.dma_start(
                    out=wqk_all[:, 1:3, :],
                    in_=wqk[128:384, :].rearrange("(k p) c -> p k c", k=2),
                )
                nc.scalar.dma_start(
                    out=xT_all[:, 1, 0:1024], in_=xT[128:256, 0:1024]
                )
                nc.sync.dma_start(
                    out=wqk_all[:, 3:5, :],
                    in_=wqk[384:640, :].rearrange("(k p) c -> p k c", k=2),
                )
                nc.scalar.dma_start(
                    out=xT_all[:, 2, 0:1024], in_=xT[256:384, 0:1024]
                )
                nc.sync.dma_start(
                    out=wqk_all[:, 5:KT, :],
                    in_=wqk[640:D, :].rearrange("(k p) c -> p k c", k=3),
                )
                for kt in range(3, KT):
                    nc.scalar.dma_start(
                        out=xT_all[:, kt, 0:1024],
                        in_=xT[kt * 128 : (kt + 1) * 128, 0:1024],
                    )
                for kt in range(KT):
                    eng = [nc.sync, nc.scalar][kt % 2]
                    eng.dma_start(
                        out=xT_all[:, kt, 1024:2048],
                        in_=xT[kt * 128 : (kt + 1) * 128, 1024:2048],
                    )
                nc.scalar.dma_start(
                    out=wv_all,
                    in_=wv[:, :].rearrange("(k p) c -> p k c", k=KT),
                )
                for ft in range(2):
                    nc.scalar.dma_start(
                        out=wp_sb[:, ft, :], in_=wp[ft * 128 : (ft + 1) * 128, :]
                    )
                nc.scalar.dma_start(out=tri_sb, in_=tric[:, :, :])
                for tt in range(TT):
                    nc.gpsimd.memset(V1_sb[tt][:, :, 64:65], 1.0)

                with tc.tile_pool(name="psA1", bufs=4, space="PSUM") as psa1:
                    # wave1: ALL m-tiles x T-cols 0..1023, kt-outer so the
                    # matmul stream tracks the per-kt x DMA arrivals. 8
                    # concurrent psum groups: 4 from psa1 plus the 4 halves
                    # of the 2 psS ring slots (idle until the first S step),
                    # so the PE is ~fully busy during the paced x load.
                    wave1 = [(0, 0), (2, 0), (0, 1), (2, 1)]
                    wave1b = [(1, 0), (3, 0), (1, 1), (3, 1)]
                    pstiles = {}
                    for mt, g in wave1:
                        pstiles[(mt, g)] = psa1.tile(
                            [128, 512], F32, tag="qkps", name=f"qk{mt}{g}"
                        )
                    sslots = [
                        pss_pool.tile([128, 2, 512], F32, tag="s", name=f"w1b{i}")
                        for i in range(2)
                    ]
                    for i, (mt, g) in enumerate(wave1b):
                        pstiles[(mt, g)] = sslots[i // 2][:, i % 2, :]
                    for kt in range(KT):
                        for mt, g in wave1 + wave1b:
                            nc.tensor.matmul(
                                pstiles[(mt, g)],
                                wqk_all[:, kt, mt * 128 : (mt + 1) * 128],
                                xT_all[:, kt, g * 512 : (g + 1) * 512],
                                start=(kt == 0),
                                stop=(kt == KT - 1),
                            )
                    for i, (mt, g) in enumerate(wave1 + wave1b):
                        # split copies across Act (idle before the first exp)
                        # and DVE
                        eng = nc.scalar.copy if i % 2 == 0 else (
                            lambda o, s: nc.vector.tensor_copy(o, s)
                        )
                        eng(
                            qkT_sb[mt][:, g * 512 : (g + 1) * 512],
                            pstiles[(mt, g)],
                        )
                        qk_copied.add((mt, g))
                    ov_emit(6)
                    # remaining qk groups, kt-inner, ordered to unlock S
                    # items (qc0 p1 first, then qc3 p0, ...) as fast as Act
                    # can consume them
                    rest = [
                        (0, 3), (2, 2), (2, 3), (3, 2),
                        (3, 3), (1, 3), (0, 2), (1, 2),
                    ]
                    for gi, (mt, g) in enumerate(rest):
                        ps = psa1.tile(
                            [128, 512], F32, tag="qkps", name=f"qk{mt}{g}"
                        )
                        for kt in range(KT):
                            nc.tensor.matmul(
                                ps,
                                wqk_all[:, kt, mt * 128 : (mt + 1) * 128],
                                xT_all[:, kt, g * 512 : (g + 1) * 512],
                                start=(kt == 0),
                                stop=(kt == KT - 1),
                            )
                        nc.vector.tensor_copy(
                            qkT_sb[mt][:, g * 512 : (g + 1) * 512], ps
                        )
                        qk_copied.add((mt, g))
                        ov_emit(1 if gi < 4 else OV_SPG)

                with tc.tile_pool(name="psB", bufs=2, space="PSUM") as psb:
                    for tt in range(TT):
                        psv = psb.tile([128, 256], F32, tag="v", name=f"v{tt}")
                        for kt in range(KT):
                            nc.tensor.matmul(
                                psv,
                                xT_all[:, kt, tt * 128 : (tt + 1) * 128],
                                wv_all[:, kt, :],
                                start=(kt == 0),
                                stop=(kt == KT - 1),
                            )
                        nc.vector.tensor_copy(V1_sb[tt][:, :, 0:64], psv)
                        ov_emit(OV_SPG)

            # ---------------- phase B: PV / normalize / proj ----------------
            pso_tiles = {}

            def emit_PV(j, p):
                qc, regs = steps[j]
                pt = pt_tiles[(j, p)]
                # psum start-zeroing is bank-granular: exactly ONE matmul per
                # O tile (the first to touch the bank: kb==0, h==0) carries
                # start=True. Its bank-wide pending-zero mark makes the other
                # heads' first writes land on zeros, and later accumulates
                # add normally -- no separate memset needed.
                for kb, rlo, rhi, slo in regs:
                    for hslot in range(2):
                        h = 2 * p + hslot
                        for qbl in range(slo // 128, (slo + rhi - rlo) // 128):
                            off = qbl * 128 - slo
                            nc.tensor.matmul(
                                pso_tiles[(qc, qbl)][:, h, :],
                                pt[:, hslot, rlo + off : rlo + off + 128],
                                V1_sb[kb][:, h, :],
                                start=(kb == 0 and h == 0),
                                stop=(kb == qc * 4 + qbl),
                                skip_group_check=True,
                            )
                if p == 1:
                    pt_tiles.pop((j, 0), None)
                    pt_tiles.pop((j, 1), None)

            def emit_norm_qb(qc, qbl, tail=False):
                # tail mode: the exp stream is done, so the Activation engine
                # is free -- normalize there to dodge the DVE queue latency
                o = pso_tiles.pop((qc, qbl))
                qbg = qc * 4 + qbl
                zc = stg.tile(
                    [128, 4, 1], F32, tag="zc", bufs=4, name=f"zc{qbg}"
                )
                tact = tail and qbl % 2 == 0
                (nc.scalar.copy if tact else nc.vector.tensor_copy)(
                    zc, o[:, :, 64:65]
                )
                rc = stg.tile(
                    [128, 4, 1], F32, tag="rc", bufs=4, name=f"rc{qbg}"
                )
                nc.vector.reciprocal_approx_fast(out=rc, in_=zc)
                for h in range(4):
                    if tact:
                        nc.scalar.activation(
                            ATn_sb[:, qbg, h * 64 : (h + 1) * 64],
                            o[:, h, 0:64],
                            mybir.ActivationFunctionType.Copy,
                            scale=rc[:, h, 0:1],
                        )
                    else:
                        nc.vector.tensor_scalar_mul(
                            ATn_sb[:, qbg, h * 64 : (h + 1) * 64],
                            o[:, h, 0:64],
                            rc[:, h, 0:1],
                        )

            def emit_transpose(qc, pair=None):
                # f-major AT^T via the DMA xbar (off all compute engines)
                if pair is None:
                    nc.sync.dma_start_transpose(
                        out=ATt_sb[qc], in_=ATn_sb[:, qc * 4 : (qc + 1) * 4, :]
                    )
                else:
                    nc.sync.dma_start_transpose(
                        out=ATt_sb[qc][:, 2 * pair : 2 * pair + 2, :, :],
                        in_=ATn_sb[:, qc * 4 + 2 * pair : qc * 4 + 2 * pair + 2, :],
                    )

            ost_n = [0]
            dma_queue = []
            stg2_box = []

            def flush_dmas():
                while dma_queue:
                    ost, w, row, cols = dma_queue.pop(0)
                    dma0 = [nc.sync, nc.scalar][ost_n[0] % 2]
                    dma0.dma_start(
                        out=outT[row : row + 128, cols], in_=ost[:, 0, 0:w]
                    )
                    dma1 = [nc.scalar, nc.sync][ost_n[0] % 2]
                    dma1.dma_start(
                        out=outT[row + 128 : row + 256, cols],
                        in_=ost[:, 1, 0:w],
                    )

            def emit_proj(qc, jt2, qhalf=None):
                # qhalf None: full piece (q-cols 0..511 of the chunk);
                # 0/1: half piece over 2 q-blocks (tail-latency mode)
                psp = pss_pool.tile(
                    [128, 2, 512], F32, tag="s", name=f"pp{qc}_{jt2}_{qhalf}"
                )
                if qhalf is None:
                    qsl = slice(0, 4)
                    csl = slice(0, 512)
                else:
                    qsl = slice(2 * qhalf, 2 * qhalf + 2)
                    csl = slice(256 * qhalf, 256 * qhalf + 256)
                w = csl.stop - csl.start
                for sub in range(2):
                    dblk = 2 * jt2 + sub
                    for ft in range(2):
                        nc.tensor.matmul(
                            psp[:, sub, 0:w],
                            wp_sb[:, ft, dblk * 128 : (dblk + 1) * 128],
                            ATt_sb[qc][:, qsl, ft, :],
                            start=(ft == 0),
                            stop=(ft == 1),
                        )
                ost = stg2_box[0].tile(
                    [128, 2, 512], BF16, tag="ost", bufs=6,
                    name=f"ost{qc}{jt2}{qhalf}",
                )
                cols = slice(qc * 512 + csl.start, qc * 512 + csl.stop)
                # steady-state pieces stage both halves on DVE; tail
                # pieces stage half0 on DVE and half1 on Act (idle post-exp)
                # so the two copies run in parallel
                ceng = nc.vector.tensor_copy
                ceng1 = ceng
                ceng(ost[:, 0, 0:w], psp[:, 0, 0:w])
                ceng1(ost[:, 1, 0:w], psp[:, 1, 0:w])
                # defer the DMA issues one slot: by then the staging copies
                # have landed, so the DMA's SEQ-level wait is ~zero and never
                # blocks the exp stream (Act queue) behind it
                dma_queue.append(
                    (ost, w, 2 * jt2 * 128, cols)
                )
                ost_n[0] += 1

            with (
                tc.tile_pool(name="psO", bufs=4, space="PSUM") as pso_pool,
                tc.tile_pool(name="stage2", bufs=1) as stg2,
            ):
                stg2_box.append(stg2)
                proj_queue = []
                proj_cool = [0]
                S_ptr = [0]

                def alloc_O(qc):
                    for qbl in range(4):
                        pso_tiles[(qc, qbl)] = pso_pool.tile(
                            [128, 4, 65], F32, tag="o", name=f"o{qc}_{qbl}"
                        )

                while S_ptr[0] < NS and (pv_steps[S_ptr[0]], 1) in emitted:
                    S_ptr[0] += 1
                alloc_O(PV_ORDER[0])
                for jj in range(NS):
                    j = pv_steps[jj]
                    flush_dmas()
                    emit_tri(j)
                    if jj + 1 < NS:
                        emit_tri(pv_steps[jj + 1])
                    # proj before this slot's S-emissions: its psum slot (the
                    # shared "s" ring) then waits on an exp that is already
                    # done instead of one just emitted
                    if proj_queue and proj_cool[0] <= 0:
                        emit_proj(*proj_queue.pop(0))
                        proj_cool[0] = COOL
                    else:
                        proj_cool[0] -= 1
                    tgt = min(jj + DEPTH_B, NS - 1)
                    while S_ptr[0] <= tgt:
                        i = pv_steps[S_ptr[0]]
                        for p in range(2):
                            if (i, p) not in emitted:
                                emit_S(i, p)
                        S_ptr[0] += 1
                    qc, regs = steps[j]
                    emit_PV(j, 0)
                    emit_PV(j, 1)
                    # normalize each q-block at its stop step: spreads the
                    # norm work, frees O banks early (next chunk's PV never
                    # waits), and lets the tail chunk's proj start before the
                    # chunk's own last PV
                    tail = qc == LAST_QC
                    for kb in (r[0] for r in regs):
                        qbl = kb - 4 * qc
                        if qbl < 0:
                            continue
                        emit_norm_qb(qc, qbl, tail=tail)
                        if qbl == 1:
                            emit_transpose(qc, pair=0)
                            if tail:
                                for jt2 in range(4):
                                    proj_queue.append((qc, jt2, 0))
                        elif qbl == 3:
                            emit_transpose(qc, pair=1)
                            if tail:
                                while proj_queue:
                                    emit_proj(*proj_queue.pop(0))
                                for jt2 in range(4):
                                    emit_proj(qc, jt2, 1)
                            else:
                                for jt2 in range(4):
                                    proj_queue.append((qc, jt2, None))
                                nqc = PV_ORDER[PV_ORDER.index(qc) + 1]
                                alloc_O(nqc)
                while proj_queue:
                    emit_proj(*proj_queue.pop(0))
                    flush_dmas()
                flush_dmas()

    nc.finalize()
    return nc


_NC_CACHE = None


def _get_nc():
    global _NC_CACHE
    if _NC_CACHE is None:
        _NC_CACHE = build_nc()
    return _NC_CACHE


def make_in_maps(x, w_qkv, w_proj):
    import ml_dtypes

    bf16 = ml_dtypes.bfloat16
    x = np.asarray(x, dtype=np.float32).astype(bf16)
    w_qkv = np.asarray(w_qkv, dtype=np.float32).astype(bf16)
    w_proj = np.asarray(w_proj, dtype=np.float32).astype(bf16)
    tri = np.ascontiguousarray(
        np.broadcast_to(
            np.triu(np.ones((128, 128), dtype=np.float32))[:, None, :],
            (128, 2, 128),
        )
    ).astype(bf16)
    in_maps = []
    for c in range(N_CORES):
        b, g = divmod(c, 4)
        cs = 256 * g
        in_maps.append(
            {
                "xT": np.ascontiguousarray(x[b].T),
                "wqk": np.ascontiguousarray(
                    np.concatenate(
                        [w_qkv[:, cs : cs + 256], w_qkv[:, D + cs : D + cs + 256]],
                        axis=1,
                    )
                ),
                "wv": np.ascontiguousarray(w_qkv[:, 2 * D + cs : 2 * D + cs + 256]),
                "wp": np.ascontiguousarray(w_proj[cs : cs + 256, :]),
                "tric": tri,
            }
        )
    return in_maps


def assemble(results):
    out = np.empty((B, T, D), dtype=np.float32)
    for b in range(B):
        acc = results[4 * b]["outT"].astype(np.float32)
        for g in range(1, 4):
            acc = acc + results[4 * b + g]["outT"]
        out[b] = acc.T
    return out


def kernel(x, w_qkv, w_proj, trace=False):
    nc = _get_nc()
    in_maps = make_in_maps(x, w_qkv, w_proj)
    res = bass_utils.run_bass_kernel_spmd(
        nc, in_maps, core_ids=list(range(N_CORES)), trace=trace
    )
    out = assemble(res.results)
    if trace:
        kernel.last_exec_time_ns = res.exec_time_ns
        kernel.last_result = res
    return out
